# revision 47
# baseline (speedup 1.0000x reference)
"""AttentionPool Trainium2 kernel.

Computes, for x (B,T,m), W1 (m,m), W2 (m,m), vm (1,m):
    h      = tanh(x @ W1 + vm @ W2)          (B,T,m)
    scores = h @ vm[0]                       (B,T,1)
    w      = softmax(scores, axis=T)
    out    = sum(x * w, axis=T, keepdims)    (B,1,m)

Sharding: data-parallel over B across 8 NeuronCores (2 examples per core);
W1/W2/vm replicated.  Softmax needs no max pass: |scores| <= ||vm||_1 (~13
at this scale); with a fixed shift K, e' = exp(s-K) stays inside fp16 range
(overflow would need s > 13.09 > ||vm||_1), so a single streaming pass with
online accumulation of e and Z suffices; the shift cancels in acc/Z.

Per-core dataflow (chunk = 1024 rows of T, t = c*1024 + p*8 + r):
  SWDGE cast-DMA x chunk f32->fp16, two 512 KiB halves (one load feeds both
  the score path and the pooling path; setup weights go through the same
  SWDGE queue first so they are not stuck behind megabyte x transfers)
  -> PE transpose of fp16 PAIRS as fp32 words (halves transpose count):
     xts[q, r, p] holds (x[t,2q], x[t,2q+1]); h-matmuls then contract the
     full m=256 in one 128-partition sweep per parity j via strided fp16
     views, W1 pre-arranged as W1[2q+j, n]
  -> h^T matmuls (PE fp16, 512-col tiles) -> tanh + per-partition bias
     c = vm@W2 (ACT, one instr per n-half)
  -> scores: s = h^T.T @ vm per 128-t block (PE, h stationary)
  -> e16 = exp(s - K) (ACT, fp16) with fused accum_out -> Z partials
  -> pooling on PE: acc[1,m] += e16[t] * x[t,m] via lhsT=e16 column,
     accumulated in PSUM across the whole example
  -> tail: Z = partition-reduce of Z partials (PE), 1/Z (DVE), scale, DMA.
"""

import numpy as np

import concourse.bass as bass
import concourse.tile as tile
from concourse import bacc, mybir
from concourse.bass_utils import run_bass_kernel_spmd
from concourse.masks import make_identity

FP32 = mybir.dt.float32
FP16 = mybir.dt.float16
AF = mybir.ActivationFunctionType

N_CORES = 8
B = 16
B_PER_CORE = B // N_CORES  # 2
T = 8192
M = 256
P = 128
CHUNK = 1024         # t rows per chunk
NT = CHUNK // P      # 8 t-tiles (r values) per chunk
NCHUNK = T // CHUNK  # 8 chunks per example
KSUB = 2.0           # exp shift: e' = exp(s - KSUB), cancels in acc/Z


def _build_program() -> bass.Bass:
    nc = bacc.Bacc("TRN2", target_bir_lowering=False, debug=False)

    x = nc.dram_tensor("x", [B_PER_CORE, T, M], FP32, kind="ExternalInput")
    W1 = nc.dram_tensor("W1", [M, M], FP32, kind="ExternalInput")
    W2 = nc.dram_tensor("W2", [M, M], FP32, kind="ExternalInput")
    vm = nc.dram_tensor("vm", [1, M], FP32, kind="ExternalInput")
    out = nc.dram_tensor("out", [B_PER_CORE, M], FP32, kind="ExternalOutput")

    with tile.TileContext(nc) as tc:
        with (
            tc.tile_pool(name="setup", bufs=1) as setup,
            tc.tile_pool(name="xin", bufs=10) as xin_pool,
            tc.tile_pool(name="xtp", bufs=1, space="PSUM") as xtp_pool,
            tc.tile_pool(name="xts", bufs=3) as xts_pool,
            tc.tile_pool(name="hps", bufs=1, space="PSUM") as hps_pool,
            tc.tile_pool(name="hsb", bufs=3) as hsb_pool,
            tc.tile_pool(name="sps", bufs=1, space="PSUM") as sps_pool,
            tc.tile_pool(name="acc", bufs=1, space="PSUM") as acc_pool,
            tc.tile_pool(name="eee", bufs=2) as e_pool,
            tc.tile_pool(name="fin", bufs=2) as fin_pool,
        ):
            # ---------------- setup ----------------
            # All loads go through the SWDGE queue (FIFO).  The first x
            # half-chunk is issued FIRST (inside the loop below) so the PE
            # transposes start as early as possible and keep HAM warm;
            # weights follow, and the badly-shaped tiny vmt transfer
            # (128 x 8 B descriptors) goes last among the setup loads.
            identf = setup.tile([P, P], FP32)
            make_identity(nc, identf)

            w1f = setup.tile([P, 2, M], FP32)
            w1b = setup.tile([P, 2, M], FP16)
            w2f = setup.tile([P, 2, M], FP32)
            vmt_f = setup.tile([P, 2], FP32)
            vmt_b = setup.tile([P, 2], FP16)

            c_sb = setup.tile([P, 2], FP32)

            ones_col = setup.tile([P, 1], FP32)
            nc.vector.memset(ones_col, 1.0)
            kbias = setup.tile([P, 1], FP32)
            nc.vector.memset(kbias, -KSUB)
            # Preload the exp/tanh ACT table set during startup so the first
            # real tanh doesn't pay the ~2.7us table load.
            warm = setup.tile([1, 1], FP32)
            nc.scalar.activation(warm, ones_col[0:1, :], AF.Exp, bias=kbias[0:1, :])
            # Warm the PE HAM clock gate during the initial DMA wait: ~5us
            # of dummy matmuls flips the PE from 1.2 to 2.4 GHz before the
            # first real transposes, instead of ~10us into the main loop
            # (transpose-mode work does not count as HAM activity).
            # 40 spans until the first x data lands (~11us) without leaving
            # a >3.4us idle window for HAM to re-throttle in between.
            warm_ps = sps_pool.tile([P, P], FP32, tag="sps")
            for _ in range(40):
                nc.tensor.matmul(warm_ps, lhsT=identf, rhs=identf, start=True, stop=True)

            # ---------------- main loop ----------------
            # Variable chunk schedule: 512-row chunks at the pipeline fill
            # (start of example 0) and drain (end of example 1) ends so the
            # exposed first-chunk latency and last-chunk serial tail are
            # halved; 1024-row chunks in the middle for ACT efficiency.
            mid = [(i * 1024, 1024) for i in range(1, (T // 1024) - 1)]
            sched0 = [(0, 512), (512, 512)] + mid + [(T - 1024, 1024)]
            sched1 = [(0, 1024)] + mid + [(T - 1024, 512), (T - 512, 512)]
            for b in range(B_PER_CORE):
                e16 = e_pool.tile([P, NCHUNK * NT], FP16, tag="e16")
                # Pooling accumulator: 4 col-tiled groups accumulate into
                # partition rows 0/32/64/96; other rows stay at the memset 0.
                acc_ps = acc_pool.tile([P, M], FP32, tag="acc")
                nc.vector.memset(acc_ps, 0.0)

                sched = sched0 if b == 0 else sched1
                coff = 0
                for ci, (row0, rows) in enumerate(sched):
                    nt = rows // P
                    last = ci == len(sched) - 1
                    # x chunk: xin[p, r, m] = x[b, row0 + p*nt + r, m], cast
                    # f32 -> fp16 during the DMA (SWDGE).
                    xin = xin_pool.tile([P, nt, M], FP16)
                    src = x[b, row0 : row0 + rows, :].rearrange(
                        "(p r) m -> p r m", p=P
                    )
                    if b == 0 and ci == 0:
                        half = nt // 2
                        for hi in range(2):
                            nc.gpsimd.dma_start(
                                out=xin[:, hi * half : (hi + 1) * half, :],
                                in_=src[:, hi * half : (hi + 1) * half, :],
                            )
                        # weights queue behind the first x chunk; W2/vm
                        # first (the c matmuls run right after the first
                        # transposes), W1 next (needed by the h matmuls).
                        nc.gpsimd.dma_start(
                            out=w2f, in_=W2.rearrange("(a p) n -> p a n", p=P)
                        )
                        nc.gpsimd.dma_start(
                            out=vmt_f, in_=vm[0].rearrange("(a p) -> p a", p=P)
                        )
                        nc.vector.tensor_copy(vmt_b, vmt_f)
                        nc.gpsimd.dma_start(
                            out=w1f,
                            in_=W1.rearrange("(q two) n -> q two n", two=2),
                        )
                        nc.vector.tensor_copy(w1b, w1f)
                    else:
                        nc.gpsimd.dma_start(out=xin, in_=src)

                    # PE transpose of fp16 pairs as f32: xts2[w, r, p] =
                    # (x[t(p,r), 2w], x[t(p,r), 2w+1]) packed in one f32.
                    # Two half tiles so next chunk's transposes overlap this
                    # chunk's PSUM->SBUF drain.
                    xin32 = xin.bitcast(FP32)  # [P, nt, 128]
                    xts2 = xts_pool.tile([P, nt, P], FP32, tag="xts")
                    nh2 = nt // 2
                    for hi in range(2):
                        xtp2 = xtp_pool.tile([P, nh2, P], FP32, tag=f"xtp{hi}")
                        for r in range(nh2):
                            nc.tensor.transpose(
                                xtp2[:, r, :], xin32[:, hi * nh2 + r, :], identf
                            )
                        nc.vector.tensor_copy(
                            xts2[:, hi * nh2 : (hi + 1) * nh2, :], xtp2
                        )
                    # fp16 parity views: xtsr[w, j, r, p] = x[t(p,r), 2w+j]
                    xtsr = xts2.bitcast(FP16).rearrange(
                        "w r (p two) -> w two r p", two=2
                    )

                    if b == 0 and ci == 0:
                        # c = vm @ W2, transposed: c_sb[p, nh] = c[nh*128+p].
                        # Emitted after the first transposes so these fp32
                        # matmuls (waiting on W2/vm) don't head-of-line
                        # block the PE queue.
                        c_ps = sps_pool.tile([P, 2], FP32, tag="sps")
                        for nh in range(2):
                            for mh in range(2):
                                nc.tensor.matmul(
                                    c_ps[:, nh : nh + 1],
                                    lhsT=w2f[:, mh, nh * P : (nh + 1) * P],
                                    rhs=vmt_f[:, mh : mh + 1],
                                    start=(mh == 0),
                                    stop=(mh == 1),
                                )
                        nc.vector.tensor_copy(c_sb, c_ps)

                    # h^T = W1.T @ x^T per n-half; full m-contraction per
                    # parity j; 512-col tiles (PSUM bank limit)
                    hsb = []
                    for nh in range(2):
                        hps = hps_pool.tile([P, rows], FP32, tag=f"hps{nh}")
                        for th in range(rows // 512):
                            for j in range(2):
                                nc.tensor.matmul(
                                    hps[:, th * 512 : (th + 1) * 512],
                                    lhsT=w1b[:, j, nh * P : (nh + 1) * P],
                                    rhs=xtsr[:, j, th * 4 : (th + 1) * 4, :],
                                    start=(j == 0),
                                    stop=(j == 1),
                                )
                        # tanh with per-partition bias c, one instr per n-half
                        hsb_nh = hsb_pool.tile([P, rows], FP16, tag=f"hsb{nh}")
                        nc.scalar.activation(
                            hsb_nh,
                            hps,
                            AF.Tanh,
                            bias=c_sb[:, nh : nh + 1],
                        )
                        hsb.append(hsb_nh)

                    # scores: s[q, r] for t = row0 + q*nt + r (t-partitioned)
                    sps = sps_pool.tile([P, nt], FP32, tag="sps")
                    for r in range(nt):
                        for nh in range(2):
                            nc.tensor.matmul(
                                sps[:, r : r + 1],
                                lhsT=hsb[nh][:, r * P : (r + 1) * P],
                                rhs=vmt_b[:, nh : nh + 1],
                                start=(nh == 0),
                                stop=(nh == 1),
                            )

                    # e' = exp(s - K) in fp16
                    nc.scalar.activation(
                        e16[:, coff : coff + nt],
                        sps,
                        AF.Exp,
                        bias=kbias,
                    )

                    # pooling: acc[32g, m] += sum_p e'[p, coff+r] * x[p, r, m]
                    # with g = r%4 -> four col-tiled groups run concurrently
                    # in distinct 32-column strips of the PE array.
                    for r in range(nt):
                        g = r % 4
                        nc.tensor.matmul(
                            acc_ps[32 * g : 32 * g + 1, :],
                            lhsT=e16[:, coff + r : coff + r + 1],
                            rhs=xin[:, r],
                            start=(ci == 0 and r < 4),
                            stop=(last and r >= nt - 4),
                            skip_group_check=True,
                            tile_position=(0, 32 * g),
                        )
                    coff += nt

                # ---- finalize example ----
                if b == B_PER_CORE - 1:
                    # Keep the PE warm through the final DVE/ACT-bound
                    # stretch so the tail matmuls run at 2.4 GHz.
                    tail_warm = sps_pool.tile([P, P], FP32, tag="sps")
                    for _ in range(8):
                        nc.tensor.matmul(
                            tail_warm, lhsT=identf, rhs=identf, start=True, stop=True
                        )
                # Z: free-dim reduce of the e' table (f32 accumulate), then
                # partition reduce on PE, reciprocal.  Using the same fp16
                # e' values the pooling used keeps acc/Z exact.
                z_red = fin_pool.tile([P, 1], FP32)
                nc.vector.reduce_sum(z_red, e16, axis=mybir.AxisListType.X)
                z_ps = sps_pool.tile([1, 1], FP32, tag="sps")
                nc.tensor.matmul(z_ps, lhsT=z_red, rhs=ones_col, start=True, stop=True)
                rz = fin_pool.tile([1, 1], FP32)
                nc.vector.reciprocal(rz, z_ps)
                # combine the 4 col-tiled pooling groups: rows 0/32/64/96
                # hold partials, the rest are zero from the memset; one
                # 97-partition reduce lands the m-contiguous output row.
                acc_sb = fin_pool.tile([97, M], FP32)
                nc.vector.tensor_copy(acc_sb, acc_ps[0:97, :])
                osum_ps = sps_pool.tile([1, M], FP32, tag="sps")
                nc.tensor.matmul(
                    osum_ps,
                    lhsT=ones_col[0:97, :],
                    rhs=acc_sb,
                    start=True,
                    stop=True,
                )
                outsb = fin_pool.tile([1, M], FP32)
                nc.vector.tensor_scalar_mul(outsb, osum_ps, rz)
                # SWDGE + single packet for the tiny contiguous output row:
                # a cold 16-engine completion was measured at ~10us.
                nc.gpsimd.dma_start(
                    out=out[b : b + 1, :], in_=outsb, single_packet=True
                )

    return nc


_PROGRAM_CACHE: list = []


def _get_program() -> bass.Bass:
    if not _PROGRAM_CACHE:
        nc = _build_program()
        nc.finalize()
        _PROGRAM_CACHE.append(nc)
    return _PROGRAM_CACHE[0]


def kernel(x, W1, W2, vm):
    x = np.ascontiguousarray(x, dtype=np.float32)
    W1 = np.ascontiguousarray(W1, dtype=np.float32)
    W2 = np.ascontiguousarray(W2, dtype=np.float32)
    vm = np.ascontiguousarray(vm, dtype=np.float32)

    nc = _get_program()
    core_ids = list(range(N_CORES))
    in_maps = [
        {
            "x": x[i * B_PER_CORE : (i + 1) * B_PER_CORE],
            "W1": W1,
            "W2": W2,
            "vm": vm,
        }
        for i in range(N_CORES)
    ]
    res = run_bass_kernel_spmd(nc, in_maps, core_ids)
    out = np.concatenate([res.results[i]["out"] for i in range(N_CORES)], axis=0)
    return out.reshape(B, 1, M)


# revision 49
# speedup vs baseline: 1.0808x; 1.0808x over previous
"""AttentionPool Trainium2 kernel.

Computes, for x (B,T,m), W1 (m,m), W2 (m,m), vm (1,m):
    h      = tanh(x @ W1 + vm @ W2)          (B,T,m)
    scores = h @ vm[0]                       (B,T,1)
    w      = softmax(scores, axis=T)
    out    = sum(x * w, axis=T, keepdims)    (B,1,m)

Sharding: data-parallel over B across 8 NeuronCores (2 examples per core);
W1/W2/vm replicated.  Softmax needs no max pass: |scores| <= ||vm||_1 (~13
at this scale); with a fixed shift K, e' = exp(s-K) stays inside fp16 range
(overflow would need s > 13.09 > ||vm||_1), so a single streaming pass with
online accumulation of e and Z suffices; the shift cancels in acc/Z.

Per-core dataflow (chunk = 1024 rows of T, t = c*1024 + p*8 + r):
  SWDGE cast-DMA x chunk f32->fp16, two 512 KiB halves (one load feeds both
  the score path and the pooling path; setup weights go through the same
  SWDGE queue first so they are not stuck behind megabyte x transfers)
  -> PE transpose of fp16 PAIRS as fp32 words (halves transpose count):
     xts[q, r, p] holds (x[t,2q], x[t,2q+1]); h-matmuls then contract the
     full m=256 in one 128-partition sweep per parity j via strided fp16
     views, W1 pre-arranged as W1[2q+j, n]
  -> h^T matmuls (PE fp16, 512-col tiles) -> tanh + per-partition bias
     c = vm@W2 (ACT, one instr per n-half)
  -> scores: s = h^T.T @ vm per 128-t block (PE, h stationary)
  -> e16 = exp(s - K) (ACT, fp16) with fused accum_out -> Z partials
  -> pooling on PE: acc[1,m] += e16[t] * x[t,m] via lhsT=e16 column,
     accumulated in PSUM across the whole example
  -> tail: Z = partition-reduce of Z partials (PE), 1/Z (DVE), scale, DMA.
"""

import numpy as np

import concourse.bass as bass
import concourse.tile as tile
from concourse import bacc, mybir
from concourse.bass_utils import run_bass_kernel_spmd
from concourse.masks import make_identity

FP32 = mybir.dt.float32
FP16 = mybir.dt.float16
AF = mybir.ActivationFunctionType

N_CORES = 8
B = 16
B_PER_CORE = B // N_CORES  # 2
T = 8192
M = 256
P = 128
CHUNK = 1024         # t rows per chunk
NT = CHUNK // P      # 8 t-tiles (r values) per chunk
NCHUNK = T // CHUNK  # 8 chunks per example
KSUB = 2.0           # exp shift: e' = exp(s - KSUB), cancels in acc/Z


def _build_program() -> bass.Bass:
    nc = bacc.Bacc("TRN2", target_bir_lowering=False, debug=False)

    x = nc.dram_tensor("x", [B_PER_CORE, T, M], FP32, kind="ExternalInput")
    W1 = nc.dram_tensor("W1", [M, M], FP32, kind="ExternalInput")
    W2 = nc.dram_tensor("W2", [M, M], FP32, kind="ExternalInput")
    vm = nc.dram_tensor("vm", [1, M], FP32, kind="ExternalInput")
    out = nc.dram_tensor("out", [B_PER_CORE, M], FP32, kind="ExternalOutput")

    with tile.TileContext(nc) as tc:
        with (
            tc.tile_pool(name="setup", bufs=1) as setup,
            tc.tile_pool(name="xin", bufs=8) as xin_pool,
            tc.tile_pool(name="xtp", bufs=1, space="PSUM") as xtp_pool,
            tc.tile_pool(name="xts", bufs=3) as xts_pool,
            tc.tile_pool(name="hps", bufs=1, space="PSUM") as hps_pool,
            tc.tile_pool(name="hsb", bufs=3) as hsb_pool,
            tc.tile_pool(name="sps", bufs=1, space="PSUM") as sps_pool,
            tc.tile_pool(name="acc", bufs=1, space="PSUM") as acc_pool,
            tc.tile_pool(name="eee", bufs=2) as e_pool,
            tc.tile_pool(name="fin", bufs=2) as fin_pool,
        ):
            # ---------------- setup ----------------
            # All loads go through the SWDGE queue (FIFO).  The first x
            # half-chunk is issued FIRST (inside the loop below) so the PE
            # transposes start as early as possible and keep HAM warm;
            # weights follow, and the badly-shaped tiny vmt transfer
            # (128 x 8 B descriptors) goes last among the setup loads.
            identf = setup.tile([P, P], FP32)
            make_identity(nc, identf)

            w1f = setup.tile([P, 2, M], FP32)
            w1b = setup.tile([P, 2, M], FP16)
            w2f = setup.tile([P, 2, M], FP32)
            vmt_f = setup.tile([P, 2], FP32)
            vmt_b = setup.tile([P, 2], FP16)

            c_sb = setup.tile([P, 2], FP32)

            ones_col = setup.tile([P, 1], FP32)
            nc.vector.memset(ones_col, 1.0)
            kbias = setup.tile([P, 1], FP32)
            nc.vector.memset(kbias, -KSUB)
            # Preload the exp/tanh ACT table set during startup so the first
            # real tanh doesn't pay the ~2.7us table load.
            warm = setup.tile([1, 1], FP32)
            nc.scalar.activation(warm, ones_col[0:1, :], AF.Exp, bias=kbias[0:1, :])
            # Warm the PE HAM clock gate during the initial DMA wait: ~5us
            # of dummy matmuls flips the PE from 1.2 to 2.4 GHz before the
            # first real transposes, instead of ~10us into the main loop
            # (transpose-mode work does not count as HAM activity).
            warm_ps = sps_pool.tile([P, P], FP32, tag="sps")
            for _ in range(24):
                nc.tensor.matmul(warm_ps, lhsT=identf, rhs=identf, start=True, stop=True)

            # ---------------- main loop ----------------
            # Variable chunk schedule: 512-row chunks at the pipeline fill
            # (start of example 0) and drain (end of example 1) ends so the
            # exposed first-chunk latency and last-chunk serial tail are
            # halved; 1024-row chunks in the middle for ACT efficiency.
            mid = [(i * 1024, 1024) for i in range(1, (T // 1024) - 1)]
            sched0 = [(0, 512), (512, 512)] + mid + [(T - 1024, 1024)]
            sched1 = [(0, 1024)] + mid + [(T - 1024, 512), (T - 512, 512)]
            for b in range(B_PER_CORE):
                e16 = e_pool.tile([P, NCHUNK * NT], FP16, tag="e16")
                # Pooling accumulator: 4 col-tiled groups accumulate into
                # partition rows 0/32/64/96; other rows stay at the memset 0.
                acc_ps = acc_pool.tile([P, M], FP32, tag="acc")
                nc.vector.memset(acc_ps, 0.0)

                sched = sched0 if b == 0 else sched1
                coff = 0
                for ci, (row0, rows) in enumerate(sched):
                    nt = rows // P
                    last = ci == len(sched) - 1
                    # x chunk: xin[p, r, m] = x[b, row0 + p*nt + r, m], cast
                    # f32 -> fp16 during the DMA (SWDGE).
                    xin = xin_pool.tile([P, nt, M], FP16)
                    src = x[b, row0 : row0 + rows, :].rearrange(
                        "(p r) m -> p r m", p=P
                    )
                    if b == 0 and ci == 0:
                        half = nt // 2
                        for hi in range(2):
                            nc.gpsimd.dma_start(
                                out=xin[:, hi * half : (hi + 1) * half, :],
                                in_=src[:, hi * half : (hi + 1) * half, :],
                            )
                        # weights queue behind the first x chunk; W2/vm
                        # first (the c matmuls run right after the first
                        # transposes), W1 next (needed by the h matmuls).
                        nc.gpsimd.dma_start(
                            out=w2f, in_=W2.rearrange("(a p) n -> p a n", p=P)
                        )
                        nc.gpsimd.dma_start(
                            out=vmt_f, in_=vm[0].rearrange("(a p) -> p a", p=P)
                        )
                        nc.vector.tensor_copy(vmt_b, vmt_f)
                        nc.gpsimd.dma_start(
                            out=w1f,
                            in_=W1.rearrange("(q two) n -> q two n", two=2),
                        )
                        nc.vector.tensor_copy(w1b, w1f)
                    else:
                        nc.gpsimd.dma_start(out=xin, in_=src)

                    # PE transpose of fp16 pairs as f32: xts2[w, r, p] =
                    # (x[t(p,r), 2w], x[t(p,r), 2w+1]) packed in one f32.
                    # Two half tiles so next chunk's transposes overlap this
                    # chunk's PSUM->SBUF drain.
                    xin32 = xin.bitcast(FP32)  # [P, nt, 128]
                    xts2 = xts_pool.tile([P, nt, P], FP32, tag="xts")
                    nh2 = nt // 2
                    for hi in range(2):
                        xtp2 = xtp_pool.tile([P, nh2, P], FP32, tag=f"xtp{hi}")
                        for r in range(nh2):
                            nc.tensor.transpose(
                                xtp2[:, r, :], xin32[:, hi * nh2 + r, :], identf
                            )
                        nc.vector.tensor_copy(
                            xts2[:, hi * nh2 : (hi + 1) * nh2, :], xtp2
                        )
                    # fp16 parity views: xtsr[w, j, r, p] = x[t(p,r), 2w+j]
                    xtsr = xts2.bitcast(FP16).rearrange(
                        "w r (p two) -> w two r p", two=2
                    )

                    if b == 0 and ci == 0:
                        # c = vm @ W2, transposed: c_sb[p, nh] = c[nh*128+p].
                        # Emitted after the first transposes so these fp32
                        # matmuls (waiting on W2/vm) don't head-of-line
                        # block the PE queue.
                        c_ps = sps_pool.tile([P, 2], FP32, tag="sps")
                        for nh in range(2):
                            for mh in range(2):
                                nc.tensor.matmul(
                                    c_ps[:, nh : nh + 1],
                                    lhsT=w2f[:, mh, nh * P : (nh + 1) * P],
                                    rhs=vmt_f[:, mh : mh + 1],
                                    start=(mh == 0),
                                    stop=(mh == 1),
                                )
                        nc.vector.tensor_copy(c_sb, c_ps)

                    # h^T = W1.T @ x^T per n-half; full m-contraction per
                    # parity j.  One PSUM tile per 512-col th-piece (same
                    # 4-bank footprint) so each tanh frees its tile early
                    # and the next chunk's h-matmuls don't wait on a whole
                    # [128,1024] activation.
                    hsb = []
                    for nh in range(2):
                        hsb_nh = hsb_pool.tile([P, rows], FP16, tag=f"hsb{nh}")
                        for th in range(rows // 512):
                            hps = hps_pool.tile(
                                [P, 512], FP32, tag=f"hps{nh}{th}"
                            )
                            for j in range(2):
                                nc.tensor.matmul(
                                    hps,
                                    lhsT=w1b[:, j, nh * P : (nh + 1) * P],
                                    rhs=xtsr[:, j, th * 4 : (th + 1) * 4, :],
                                    start=(j == 0),
                                    stop=(j == 1),
                                )
                            # tanh with per-partition bias c
                            nc.scalar.activation(
                                hsb_nh[:, th * 512 : (th + 1) * 512],
                                hps,
                                AF.Tanh,
                                bias=c_sb[:, nh : nh + 1],
                            )
                        hsb.append(hsb_nh)

                    # scores: s[q, r] for t = row0 + q*nt + r (t-partitioned)
                    sps = sps_pool.tile([P, nt], FP32, tag="sps")
                    for r in range(nt):
                        for nh in range(2):
                            nc.tensor.matmul(
                                sps[:, r : r + 1],
                                lhsT=hsb[nh][:, r * P : (r + 1) * P],
                                rhs=vmt_b[:, nh : nh + 1],
                                start=(nh == 0),
                                stop=(nh == 1),
                            )

                    # e' = exp(s - K) in fp16
                    nc.scalar.activation(
                        e16[:, coff : coff + nt],
                        sps,
                        AF.Exp,
                        bias=kbias,
                    )

                    # pooling: acc[32g, m] += sum_p e'[p, coff+r] * x[p, r, m]
                    # with g = r%4 -> four col-tiled groups run concurrently
                    # in distinct 32-column strips of the PE array.
                    for r in range(nt):
                        g = r % 4
                        nc.tensor.matmul(
                            acc_ps[32 * g : 32 * g + 1, :],
                            lhsT=e16[:, coff + r : coff + r + 1],
                            rhs=xin[:, r],
                            start=(ci == 0 and r < 4),
                            stop=(last and r >= nt - 4),
                            skip_group_check=True,
                            tile_position=(0, 32 * g),
                        )
                    coff += nt

                # ---- finalize example ----
                # Z: free-dim reduce of the e' table (f32 accumulate), then
                # partition reduce on PE, reciprocal.  Using the same fp16
                # e' values the pooling used keeps acc/Z exact.
                z_red = fin_pool.tile([P, 1], FP32)
                nc.vector.reduce_sum(z_red, e16, axis=mybir.AxisListType.X)
                z_ps = sps_pool.tile([1, 1], FP32, tag="sps")
                nc.tensor.matmul(z_ps, lhsT=z_red, rhs=ones_col, start=True, stop=True)
                rz = fin_pool.tile([1, 1], FP32)
                nc.vector.reciprocal(rz, z_ps)
                # combine the 4 col-tiled pooling groups: rows 0/32/64/96
                # hold partials, the rest are zero from the memset; one
                # 97-partition reduce lands the m-contiguous output row.
                acc_sb = fin_pool.tile([97, M], FP32)
                nc.vector.tensor_copy(acc_sb, acc_ps[0:97, :])
                osum_ps = sps_pool.tile([1, M], FP32, tag="sps")
                nc.tensor.matmul(
                    osum_ps,
                    lhsT=ones_col[0:97, :],
                    rhs=acc_sb,
                    start=True,
                    stop=True,
                )
                outsb = fin_pool.tile([1, M], FP32)
                nc.vector.tensor_scalar_mul(outsb, osum_ps, rz)
                # SWDGE + single packet for the tiny contiguous output row:
                # a cold 16-engine completion was measured at ~10us.
                nc.gpsimd.dma_start(
                    out=out[b : b + 1, :], in_=outsb, single_packet=True
                )

    return nc


_PROGRAM_CACHE: list = []


def _get_program() -> bass.Bass:
    if not _PROGRAM_CACHE:
        nc = _build_program()
        nc.finalize()
        _PROGRAM_CACHE.append(nc)
    return _PROGRAM_CACHE[0]


def kernel(x, W1, W2, vm):
    x = np.ascontiguousarray(x, dtype=np.float32)
    W1 = np.ascontiguousarray(W1, dtype=np.float32)
    W2 = np.ascontiguousarray(W2, dtype=np.float32)
    vm = np.ascontiguousarray(vm, dtype=np.float32)

    nc = _get_program()
    core_ids = list(range(N_CORES))
    in_maps = [
        {
            "x": x[i * B_PER_CORE : (i + 1) * B_PER_CORE],
            "W1": W1,
            "W2": W2,
            "vm": vm,
        }
        for i in range(N_CORES)
    ]
    res = run_bass_kernel_spmd(nc, in_maps, core_ids)
    out = np.concatenate([res.results[i]["out"] for i in range(N_CORES)], axis=0)
    return out.reshape(B, 1, M)
